# revision 12
# baseline (speedup 1.0000x reference)
"""Distance-map kernel for Trainium2 (8 NeuronCores, Bass/Tile).

Computes, for volume x (64,128,128) and scalar threshold:
    binary   = (x >= thr)                        # {0,1}
    d_bg     = EDT(zeros at binary==1)           # dist to nearest foreground
    d_fg     = EDT(zeros at binary==0)           # dist to nearest background
    out      = 1 - (d_bg + d_fg)

Exactly one of d_bg/d_fg is 0 per voxel, so out = 1 - sqrt(d_bg^2 + d_fg^2).

Algorithm: separable squared EDT per axis via the truncated min-plus
parabola decomposition: T steps of
    f[i] <- min(f[i], f[i-1] + (2t-1), f[i+1] + (2t-1))
give the exact parabola min-plus for displacements <= T.  The truncated
pipeline equals the exact separable EDT whenever some nearest-opposite
witness of every voxel is within per-axis displacement T.  kernel()
verifies that bound exactly on the host (scipy EDT witness indices) and
falls back to an exact host EDT otherwise (practically impossible for
~50% random volumes, whose max distance is ~2.2).

Sharding: 8 z-slabs of 8 planes, each with a replicated halo of T planes
(clamped plane indices; replicate padding is exact because any padded
candidate is strictly dominated by its source plane).  No communication.

On-chip: layout A = [partition=y(128), free=(map(2), z(8+2T), x(130))]
bf16 with sentinel guard columns; z-pass (shrinking z window) and x-pass
(guard cols) shift along free dims.  Each step is two DVE instructions:
    m = tensor_tensor.min(f<<1, f>>1)          (bf16 2x mode)
    f = scalar_tensor_tensor((m + c) min f)    (1x mode)
The y-pass cannot shift along partitions (SBUF ops only allow start
partitions 0/32/64/96), so the 16 center planes are transposed x<->y on
the otherwise-idle TensorEngine (identity matmul -> PSUM -> ACT copy),
interleaved per-map with the x/y passes so PE/ACT hide behind DVE, and
the output is written as (z, x, y) with the host undoing the transpose.
Distances stay small exact integers in bf16; sqrt/final math in fp32.
"""

import functools
import sys

import numpy as np

sys.path.insert(0, "/opt/trn_rl_repo")

Z, Y, X = 64, 128, 128
NCORES = 8
SLAB = Z // NCORES          # 8 output z-planes per core
T = 2                       # truncation radius per axis
ZH = SLAB + 2 * T           # z-planes held per core (halo included)
XG = X + 2                  # width incl. guard cols 0 and XG-1
SENT = 16384.0              # sentinel "infinity"; bf16-exact, >> 3*T^2


@functools.lru_cache(maxsize=4)
def _build(thr: float):
    import concourse.tile as tile
    from concourse import bacc, mybir

    f32 = mybir.dt.float32
    bf16 = mybir.dt.bfloat16
    Al = mybir.AluOpType

    nc = bacc.Bacc("TRN2", target_bir_lowering=False, debug=False)
    xs = nc.declare_dram_parameter("xs", [ZH, Y, X], f32, isOutput=False)
    # note: axes (z, x, y) — host transposes back
    out = nc.declare_dram_parameter("out", [SLAB, X, Y], f32, isOutput=True)

    zc = T  # center slab planes [T, T+SLAB)

    with tile.TileContext(nc) as tc:
        with (
            tc.tile_pool(name="p", bufs=1) as pool,
            tc.tile_pool(name="ps", bufs=4, space="PSUM") as psum,
        ):
            xin = pool.tile([Y, ZH, X], f32, tag="xin")
            F = pool.tile([Y, 2, ZH, XG], bf16, tag="F")       # layout A
            M = pool.tile([Y, 2, ZH, XG], bf16, tag="M")
            FB = pool.tile([X, 2, SLAB, XG], bf16, tag="FB")   # layout B (y in free)
            MB = pool.tile([X, 2, SLAB, XG], bf16, tag="MB")
            DD = pool.tile([X, SLAB, Y], f32, tag="DD")
            IDN = pool.tile([Y, X], bf16, tag="IDN")           # identity for PE transpose

            # ---- load input slab (z y x) -> [y, z, x] in halves; binarize
            #      each half as soon as it lands ----
            src = xs[:].rearrange("z y x -> y z x")
            hz = ZH // 2
            for h0, h1 in ((0, hz), (hz, ZH)):
                nc.sync.dma_start(xin[:, h0:h1, :], src[:, h0:h1, :])
            # guards + identity while DMA runs
            nc.gpsimd.memset(F[:, :, :, 0:1], SENT)
            nc.gpsimd.memset(F[:, :, :, XG - 1 : XG], SENT)
            nc.gpsimd.memset(FB[:, :, :, 0:1], SENT)
            nc.gpsimd.memset(FB[:, :, :, XG - 1 : XG], SENT)
            ones = nc.const_aps.tensor(1.0, (Y, X), bf16)
            nc.gpsimd.affine_select(
                IDN[:], ones, [[1, X]], Al.is_equal, 0.0, base=0, channel_multiplier=-1
            )
            for h0, h1 in ((0, hz), (hz, ZH)):
                # map 1 (fg EDT): zeros at bg -> (x >= thr) * SENT   [DVE]
                nc.vector.tensor_scalar(
                    F[:, 1, h0:h1, 1 : X + 1], xin[:, h0:h1, :],
                    float(thr), SENT, op0=Al.is_ge, op1=Al.mult,
                )
                # map 0 (bg EDT): zeros at fg -> (x < thr) * SENT   [GpSimd]
                nc.gpsimd.tensor_scalar(
                    F[:, 0, h0:h1, 1 : X + 1], xin[:, h0:h1, :],
                    float(thr), SENT, op0=Al.is_lt, op1=Al.mult,
                )

            def step(f_c, f_lo, f_hi, m, c):
                """f_c <- min(f_c, f_lo + c, f_hi + c), three DVE instructions
                (TT min @2x, in-place TS add @4x, TT min @2x) — cheaper than
                one 1x-mode scalar_tensor_tensor."""
                nc.vector.tensor_tensor(m, f_lo, f_hi, op=Al.min)
                nc.vector.tensor_scalar(m, m, c, None, op0=Al.add)
                nc.vector.tensor_tensor(f_c, f_c, m, op=Al.min)

            def zpass(mp):
                for t in range(1, T + 1):
                    zr = ZH - 2 * t
                    z0 = t
                    step(
                        F[:, mp, z0 : z0 + zr, :],
                        F[:, mp, z0 - 1 : z0 - 1 + zr, :],
                        F[:, mp, z0 + 1 : z0 + 1 + zr, :],
                        M[:, mp, z0 : z0 + zr, :],
                        float(2 * t - 1),
                    )

            def xpass(mp):
                for t in range(1, T + 1):
                    step(
                        F[:, mp, zc : zc + SLAB, 1 : X + 1],
                        F[:, mp, zc : zc + SLAB, 0:X],
                        F[:, mp, zc : zc + SLAB, 2 : X + 2],
                        M[:, mp, zc : zc + SLAB, 1 : X + 1],
                        float(2 * t - 1),
                    )

            def transpose(mp):
                for z in range(SLAB):
                    pt = psum.tile([X, Y], bf16, tag="pt")
                    nc.tensor.transpose(pt[:], F[:, mp, zc + z, 1 : X + 1], IDN[:])
                    nc.scalar.copy(FB[:, mp, z, 1 : X + 1], pt[:])

            def ypass(mp):
                for t in range(1, T + 1):
                    step(
                        FB[:, mp, :, 1 : X + 1],
                        FB[:, mp, :, 0:X],
                        FB[:, mp, :, 2 : X + 2],
                        MB[:, mp, :, 1 : X + 1],
                        float(2 * t - 1),
                    )

            # interleave so PE/ACT transposes hide behind DVE passes;
            # map 1 first (its indicator is built on DVE, map 0's on GpSimd)
            zpass(1)
            xpass(1)
            transpose(1)
            zpass(0)
            xpass(0)
            transpose(0)
            ypass(1)
            ypass(0)

            # ---- finalize: out = 1 - sqrt(d_bg^2 + d_fg^2), in z-halves ----
            dst = out[:].rearrange("z x y -> x z y")
            hh = SLAB // 2
            for h0, h1 in ((0, hh), (hh, SLAB)):
                s = MB[:, 0, h0:h1, 1 : X + 1]
                nc.vector.tensor_tensor(
                    s, FB[:, 0, h0:h1, 1 : X + 1], FB[:, 1, h0:h1, 1 : X + 1],
                    op=Al.add,
                )
                nc.scalar.sqrt(DD[:, h0:h1, :], s)
                nc.vector.tensor_scalar(
                    DD[:, h0:h1, :], DD[:, h0:h1, :], -1.0, 1.0,
                    op0=Al.mult, op1=Al.add,
                )
                nc.sync.dma_start(dst[:, h0:h1, :], DD[:, h0:h1, :])

    nc.compile()
    return nc


def _slab_inputs(x: np.ndarray) -> list[dict[str, np.ndarray]]:
    in_maps = []
    for c in range(NCORES):
        idx = np.clip(np.arange(c * SLAB - T, c * SLAB + SLAB + T), 0, Z - 1)
        in_maps.append({"xs": np.ascontiguousarray(x[idx], dtype=np.float32)})
    return in_maps


def _assemble(results) -> np.ndarray:
    # per-core output is (z, x, y); transpose back to (z, y, x)
    slabs = [results[c]["out"].transpose(0, 2, 1) for c in range(NCORES)]
    return np.ascontiguousarray(np.concatenate(slabs, axis=0), dtype=np.float32)


def _run(x: np.ndarray, thr: float, trace: bool = False):
    from concourse.bass_utils import run_bass_kernel_spmd

    nc = _build(float(thr))
    res = run_bass_kernel_spmd(nc, _slab_inputs(x), list(range(NCORES)), trace=trace)
    return _assemble(res.results), res


def _check_t_sufficient(x: np.ndarray, thr: float) -> bool:
    """True iff every voxel has a nearest-opposite-class witness with
    per-axis displacement <= T (exact sufficiency for the truncated EDT)."""
    from scipy import ndimage

    fg = x >= thr
    if fg.all() or (~fg).all():
        return False
    for mask in (~fg, fg):
        _, idx = ndimage.distance_transform_edt(mask, return_indices=True)
        for ax in range(3):
            g = np.arange(x.shape[ax]).reshape(
                [-1 if a == ax else 1 for a in range(3)]
            )
            if np.abs(idx[ax] - g).max() > T:
                return False
    return True


def _reference_numpy(x: np.ndarray, thr: float) -> np.ndarray:
    """Exact fallback (host)."""
    from scipy import ndimage

    fg = x >= thr
    d_bg = ndimage.distance_transform_edt(~fg) if not fg.all() else np.zeros_like(x)
    d_fg = ndimage.distance_transform_edt(fg) if fg.any() else np.zeros_like(x)
    return (1.0 - (d_bg + d_fg)).astype(np.float32)


def kernel(x: np.ndarray, threshold: np.ndarray) -> np.ndarray:
    x = np.asarray(x, dtype=np.float32)
    thr = float(np.asarray(threshold))
    if not _check_t_sufficient(x, thr):
        return _reference_numpy(x, thr)
    full, _ = _run(x, thr, trace=False)
    return full


# revision 13
# speedup vs baseline: 1.5179x; 1.5179x over previous
"""Distance-map kernel for Trainium2 (8 NeuronCores, Bass/Tile).

Computes, for volume x (64,128,128) and scalar threshold:
    binary   = (x >= thr)                        # {0,1}
    d_bg     = EDT(zeros at binary==1)           # dist to nearest foreground
    d_fg     = EDT(zeros at binary==0)           # dist to nearest background
    out      = 1 - (d_bg + d_fg)

Exactly one of d_bg/d_fg is 0 per voxel, so out = 1 - sqrt(d_bg^2 + d_fg^2).

Algorithm: separable squared EDT per axis via the truncated min-plus
parabola decomposition: T steps of
    f[i] <- min(f[i], f[i-1] + (2t-1), f[i+1] + (2t-1))
give the exact parabola min-plus for displacements <= T.  The truncated
pipeline equals the exact separable EDT whenever some nearest-opposite
witness of every voxel is within per-axis displacement T.  kernel()
verifies that bound exactly on the host (scipy EDT witness indices) and
falls back to an exact host EDT otherwise (practically impossible for
~50% random volumes, whose max distance is ~2.2).

Sharding: 8 z-slabs of 8 planes, each with a replicated halo of T planes
(clamped plane indices; replicate padding is exact because any padded
candidate is strictly dominated by its source plane).  No communication.

On-chip: layout A = [partition=y(128), free=(map(2), z(8+2T), x(130))]
bf16 with sentinel guard columns; z-pass (shrinking z window) and x-pass
(guard cols) shift along free dims.  Each step is two DVE instructions:
    m = tensor_tensor.min(f<<1, f>>1)          (bf16 2x mode)
    f = scalar_tensor_tensor((m + c) min f)    (1x mode)
The y-pass cannot shift along partitions (SBUF ops only allow start
partitions 0/32/64/96), so the 16 center planes are transposed x<->y on
the otherwise-idle TensorEngine (identity matmul -> PSUM -> ACT copy),
interleaved per-map with the x/y passes so PE/ACT hide behind DVE, and
the output is written as (z, x, y) with the host undoing the transpose.
Distances stay small exact integers in bf16; sqrt/final math in fp32.
"""

import functools
import sys

import numpy as np

sys.path.insert(0, "/opt/trn_rl_repo")

Z, Y, X = 64, 128, 128
NCORES = 8
SLAB = Z // NCORES          # 8 output z-planes per core
T = 2                       # truncation radius per axis
ZH = SLAB + 2 * T           # z-planes held per core (halo included)
XG = X + 2                  # width incl. guard cols 0 and XG-1
SENT = 16384.0              # sentinel "infinity"; bf16-exact, >> 3*T^2


@functools.lru_cache(maxsize=4)
def _build(thr: float):
    import concourse.tile as tile
    from concourse import bacc, mybir

    f32 = mybir.dt.float32
    bf16 = mybir.dt.bfloat16
    Al = mybir.AluOpType

    nc = bacc.Bacc("TRN2", target_bir_lowering=False, debug=False)
    xs = nc.declare_dram_parameter("xs", [ZH, Y, X], f32, isOutput=False)
    # note: axes (z, x, y) — host transposes back
    out = nc.declare_dram_parameter("out", [SLAB, X, Y], f32, isOutput=True)

    zc = T  # center slab planes [T, T+SLAB)

    with tile.TileContext(nc) as tc:
        with (
            tc.tile_pool(name="p", bufs=1) as pool,
            tc.tile_pool(name="ps", bufs=4, space="PSUM") as psum,
        ):
            xin = pool.tile([Y, ZH, X], f32, tag="xin")
            F = pool.tile([Y, 2, ZH, XG], bf16, tag="F")       # layout A (ping)
            G = pool.tile([Y, 2, ZH, XG], bf16, tag="G")       # layout A (pong)
            M = pool.tile([Y, 2, ZH, XG], bf16, tag="M")
            MC = pool.tile([Y, 2, ZH, XG], bf16, tag="MC")
            FB = pool.tile([X, 2, SLAB, XG], bf16, tag="FB")   # layout B (ping)
            GB = pool.tile([X, 2, SLAB, XG], bf16, tag="GB")   # layout B (pong)
            MB = pool.tile([X, 2, SLAB, XG], bf16, tag="MB")
            MCB = pool.tile([X, 2, SLAB, XG], bf16, tag="MCB")
            DD = pool.tile([X, SLAB, Y], f32, tag="DD")
            IDN = pool.tile([Y, X], bf16, tag="IDN")           # identity for PE transpose

            # ---- load input slab (z y x) -> [y, z, x] in halves; binarize
            #      each half as soon as it lands ----
            src = xs[:].rearrange("z y x -> y z x")
            hz = ZH // 2
            for h0, h1 in ((0, hz), (hz, ZH)):
                nc.sync.dma_start(xin[:, h0:h1, :], src[:, h0:h1, :])
            # guards + identity while DMA runs
            for t_ in (F, G):
                nc.vector.memset(t_[:, :, :, 0:1], SENT)
                nc.vector.memset(t_[:, :, :, XG - 1 : XG], SENT)
            for t_ in (FB, GB):
                nc.vector.memset(t_[:, :, :, 0:1], SENT)
                nc.vector.memset(t_[:, :, :, XG - 1 : XG], SENT)
            ones = nc.const_aps.tensor(1.0, (Y, X), bf16)
            nc.gpsimd.affine_select(
                IDN[:], ones, [[1, X]], Al.is_equal, 0.0, base=0, channel_multiplier=-1
            )
            for h0, h1 in ((0, hz), (hz, ZH)):
                # map 1 (fg EDT): zeros at bg -> (x >= thr) * SENT   [DVE]
                nc.vector.tensor_scalar(
                    F[:, 1, h0:h1, 1 : X + 1], xin[:, h0:h1, :],
                    float(thr), SENT, op0=Al.is_ge, op1=Al.mult,
                )
                # map 0 (bg EDT): zeros at fg -> (x < thr) * SENT
                nc.vector.tensor_scalar(
                    F[:, 0, h0:h1, 1 : X + 1], xin[:, h0:h1, :],
                    float(thr), SENT, op0=Al.is_lt, op1=Al.mult,
                )

            def step(dst, f_c, f_lo, f_hi, m, mc, c):
                """dst <- min(f_c, f_lo + c, f_hi + c), three DVE instructions
                (TT min @2x, TS add @4x, TT min @2x).  No operand aliases —
                in-place APs knock DVE down to 1x mode."""
                nc.vector.tensor_tensor(m, f_lo, f_hi, op=Al.min)
                nc.vector.tensor_scalar(mc, m, c, None, op0=Al.add)
                nc.vector.tensor_tensor(dst, f_c, mc, op=Al.min)

            assert T % 2 == 0, "ping-pong passes assume an even step count"

            def zpass(mp):
                cur, nxt = F, G
                for t in range(1, T + 1):
                    zr = ZH - 2 * t
                    z0 = t
                    step(
                        nxt[:, mp, z0 : z0 + zr, :],
                        cur[:, mp, z0 : z0 + zr, :],
                        cur[:, mp, z0 - 1 : z0 - 1 + zr, :],
                        cur[:, mp, z0 + 1 : z0 + 1 + zr, :],
                        M[:, mp, z0 : z0 + zr, :],
                        MC[:, mp, z0 : z0 + zr, :],
                        float(2 * t - 1),
                    )
                    cur, nxt = nxt, cur

            def xpass(mp):
                cur, nxt = F, G
                for t in range(1, T + 1):
                    step(
                        nxt[:, mp, zc : zc + SLAB, 1 : X + 1],
                        cur[:, mp, zc : zc + SLAB, 1 : X + 1],
                        cur[:, mp, zc : zc + SLAB, 0:X],
                        cur[:, mp, zc : zc + SLAB, 2 : X + 2],
                        M[:, mp, zc : zc + SLAB, 1 : X + 1],
                        MC[:, mp, zc : zc + SLAB, 1 : X + 1],
                        float(2 * t - 1),
                    )
                    cur, nxt = nxt, cur

            def transpose(mp):
                for z in range(SLAB):
                    pt = psum.tile([X, Y], bf16, tag="pt")
                    nc.tensor.transpose(pt[:], F[:, mp, zc + z, 1 : X + 1], IDN[:])
                    nc.scalar.copy(FB[:, mp, z, 1 : X + 1], pt[:])

            def ypass(mp):
                cur, nxt = FB, GB
                for t in range(1, T + 1):
                    step(
                        nxt[:, mp, :, 1 : X + 1],
                        cur[:, mp, :, 1 : X + 1],
                        cur[:, mp, :, 0:X],
                        cur[:, mp, :, 2 : X + 2],
                        MB[:, mp, :, 1 : X + 1],
                        MCB[:, mp, :, 1 : X + 1],
                        float(2 * t - 1),
                    )
                    cur, nxt = nxt, cur

            # interleave so PE/ACT transposes hide behind DVE passes;
            # map 1 first (its indicator is built on DVE, map 0's on GpSimd)
            zpass(1)
            xpass(1)
            transpose(1)
            zpass(0)
            xpass(0)
            transpose(0)
            ypass(1)
            ypass(0)

            # ---- finalize: out = 1 - sqrt(d_bg^2 + d_fg^2), in z-halves ----
            dst = out[:].rearrange("z x y -> x z y")
            hh = SLAB // 2
            for h0, h1 in ((0, hh), (hh, SLAB)):
                s = MB[:, 0, h0:h1, 1 : X + 1]
                nc.vector.tensor_tensor(
                    s, FB[:, 0, h0:h1, 1 : X + 1], FB[:, 1, h0:h1, 1 : X + 1],
                    op=Al.add,
                )
                nc.scalar.sqrt(DD[:, h0:h1, :], s)
                nc.vector.tensor_scalar(
                    DD[:, h0:h1, :], DD[:, h0:h1, :], -1.0, 1.0,
                    op0=Al.mult, op1=Al.add,
                )
                nc.sync.dma_start(dst[:, h0:h1, :], DD[:, h0:h1, :])

    nc.compile()
    return nc


def _slab_inputs(x: np.ndarray) -> list[dict[str, np.ndarray]]:
    in_maps = []
    for c in range(NCORES):
        idx = np.clip(np.arange(c * SLAB - T, c * SLAB + SLAB + T), 0, Z - 1)
        in_maps.append({"xs": np.ascontiguousarray(x[idx], dtype=np.float32)})
    return in_maps


def _assemble(results) -> np.ndarray:
    # per-core output is (z, x, y); transpose back to (z, y, x)
    slabs = [results[c]["out"].transpose(0, 2, 1) for c in range(NCORES)]
    return np.ascontiguousarray(np.concatenate(slabs, axis=0), dtype=np.float32)


def _run(x: np.ndarray, thr: float, trace: bool = False):
    from concourse.bass_utils import run_bass_kernel_spmd

    nc = _build(float(thr))
    res = run_bass_kernel_spmd(nc, _slab_inputs(x), list(range(NCORES)), trace=trace)
    return _assemble(res.results), res


def _check_t_sufficient(x: np.ndarray, thr: float) -> bool:
    """True iff every voxel has a nearest-opposite-class witness with
    per-axis displacement <= T (exact sufficiency for the truncated EDT)."""
    from scipy import ndimage

    fg = x >= thr
    if fg.all() or (~fg).all():
        return False
    for mask in (~fg, fg):
        _, idx = ndimage.distance_transform_edt(mask, return_indices=True)
        for ax in range(3):
            g = np.arange(x.shape[ax]).reshape(
                [-1 if a == ax else 1 for a in range(3)]
            )
            if np.abs(idx[ax] - g).max() > T:
                return False
    return True


def _reference_numpy(x: np.ndarray, thr: float) -> np.ndarray:
    """Exact fallback (host)."""
    from scipy import ndimage

    fg = x >= thr
    d_bg = ndimage.distance_transform_edt(~fg) if not fg.all() else np.zeros_like(x)
    d_fg = ndimage.distance_transform_edt(fg) if fg.any() else np.zeros_like(x)
    return (1.0 - (d_bg + d_fg)).astype(np.float32)


def kernel(x: np.ndarray, threshold: np.ndarray) -> np.ndarray:
    x = np.asarray(x, dtype=np.float32)
    thr = float(np.asarray(threshold))
    if not _check_t_sufficient(x, thr):
        return _reference_numpy(x, thr)
    full, _ = _run(x, thr, trace=False)
    return full
